# revision 9
# baseline (speedup 1.0000x reference)
"""Trainium2 Bass kernel for nn_LSA_Model (4-layer linear self-attention).

Math: each layer computes  x <- x + ((x Wq)(K^T V)) Wp  with K = x Wk, V = x Wv.
Since there is no softmax, associativity lets us avoid the [S,S] score matrix:
    KV = K^T V                  [KD, VD]   (contraction over full sequence)
    A^T = KV^T Wq^T             [VD, D]
    B  = A Wp,  C = I + B       [D, D]
    x_new = x C   (residual folded into C via the identity)

Sharding: each batch element's forward is replicated on a pair of cores
(B=4 -> 8 cores).  Inter-core collectives have a ~10us floor per call which
would dominate, so the pair does redundant compute; each core of a pair
writes a different half of the output sequence.  The second core of a pair
receives the sequence ROLLED by S/2 (the model is permutation-equivariant
over the sequence axis), so a single static NEFF can always emit its first
S/2 columns.

Layouts: on-chip state is x^T [D, S] (d-major, two 128-partition tiles) in
both fp32 (update path, via fp32r matmuls at 1 cycle/row for N>=256) and
bf16 (K|V projection path, N=128 matmuls where fp32r would be 4 cycles/row).
"""

import numpy as np
import ml_dtypes

import concourse.bacc as bacc
import concourse.mybir as mybir
import concourse.tile as tile
from concourse import masks
from concourse.bass_utils import run_bass_kernel_spmd

L, B, S, D, KD, VD = 4, 4, 4096, 256, 64, 64
H = S // 2          # output half per core
NCH = S // 128      # 32 projection chunks
NSC = S // 512      # 8 update chunks of 512
F32 = mybir.dt.float32
F32R = mybir.dt.float32r
BF16 = mybir.dt.bfloat16

_CACHE = {}


def _emit(tc, nc, xT, xTb, wkv, wqT, wp, out):
    with (
        tc.tile_pool(name="singles", bufs=1) as singles,
        tc.tile_pool(name="xf", bufs=2) as xf_pool,
        tc.tile_pool(name="xb", bufs=2) as xb_pool,
        tc.tile_pool(name="kvs", bufs=4) as kv_pool,
        tc.tile_pool(name="small", bufs=2) as small_pool,
        tc.tile_pool(name="outst", bufs=1) as out_pool,
        tc.tile_pool(name="pp", bufs=2, space="PSUM") as psum_proj,
        tc.tile_pool(name="pk", bufs=1, space="PSUM") as psum_kv,
        tc.tile_pool(name="ps", bufs=1, space="PSUM") as psum_small,
        tc.tile_pool(name="pu", bufs=2, space="PSUM") as psum_upd,
    ):
        # ---- weights to SBUF ----
        wkv_sb = singles.tile([128, L, 2, 2 * KD], BF16)
        for l in range(L):
            for t in range(2):
                nc.sync.dma_start(wkv_sb[:, l, t, :], wkv[l, t])
        # fp32r matmul operands must be produced (rounded) by a compute engine,
        # so weights bounce through an f32 staging tile into f32r tiles.
        wq_stage = singles.tile([KD, L, D], F32)
        nc.sync.dma_start(wq_stage[:], wqT.rearrange("l p f -> p l f"))
        wqT_sb = singles.tile([KD, L, D], F32R)
        nc.vector.tensor_copy(wqT_sb[:], wq_stage[:])
        wp_stage = singles.tile([VD, L, D], F32)
        nc.sync.dma_start(wp_stage[:], wp.rearrange("l p f -> p l f"))
        wp_sb = singles.tile([VD, L, D], F32R)
        nc.vector.tensor_copy(wp_sb[:], wp_stage[:])

        # identity staging: idz = [0(128) | I(128) | 0(128)]
        # C_m = B_m + idz[:, 128-m*128 : 384-m*128]
        idz = singles.tile([128, 384], F32)
        nc.gpsimd.memset(idz[:], 0.0)
        masks.make_identity(nc, idz[:, 128:256], nomemset=True)

        # ---- x state to SBUF (bf16 first: layer-0 projections need it) ----
        cur_b = [
            xb_pool.tile([128, S], BF16, tag=f"xb{t}", name=f"xb_in{t}")
            for t in range(2)
        ]
        for t in range(2):
            for sc in range(NSC):
                nc.sync.dma_start(
                    cur_b[t][:, sc * 512 : (sc + 1) * 512],
                    xTb[t, :, sc * 512 : (sc + 1) * 512],
                )
        cur_f = [
            xf_pool.tile([128, S], F32R, tag=f"xf{t}", name=f"xf_in{t}")
            for t in range(2)
        ]
        xstage = [
            singles.tile([128, S], F32, name=f"xstage{t}") for t in range(2)
        ]
        for t in range(2):
            for sc in range(NSC):
                nc.sync.dma_start(
                    xstage[t][:, sc * 512 : (sc + 1) * 512],
                    xT[t, :, sc * 512 : (sc + 1) * 512],
                )
                nc.vector.tensor_copy(
                    cur_f[t][:, sc * 512 : (sc + 1) * 512],
                    xstage[t][:, sc * 512 : (sc + 1) * 512],
                )

        for l in range(L):
            last = l == L - 1

            # ---- K|V projection (bf16) + KV = K^T V accumulation ----
            kvacc = psum_kv.tile([KD, VD], F32)
            for q in range(NCH // 4):  # 8 psum tiles, 4 chunks each
                pp = psum_proj.tile([128, 512], F32)
                for i in range(4):
                    c = q * 4 + i
                    for t in range(2):
                        nc.tensor.matmul(
                            pp[:, i * 128 : (i + 1) * 128],
                            cur_b[t][:, c * 128 : (c + 1) * 128],
                            wkv_sb[:, l, t, :],
                            start=(t == 0),
                            stop=(t == 1),
                        )
                kvt = kv_pool.tile([128, 512], BF16)
                nc.vector.tensor_copy(kvt[:], pp[:])
                for i in range(4):
                    c = q * 4 + i
                    nc.tensor.matmul(
                        kvacc[:],
                        kvt[:, i * 128 : i * 128 + KD],
                        kvt[:, i * 128 + KD : (i + 1) * 128],
                        start=(c == 0),
                        stop=(c == NCH - 1),
                    )
            kvb = small_pool.tile([KD, VD], F32R, tag="kvb")
            nc.vector.tensor_copy(kvb[:], kvacc[:])

            # ---- A^T = KV^T Wq^T ;  B = A Wp ;  C = I + B ----
            atp = psum_small.tile([VD, D], F32, tag="ps_small")
            nc.tensor.matmul(
                atp[:],
                kvb[:],
                wqT_sb[:, l, :],
                start=True,
                stop=True,
            )
            at = small_pool.tile([VD, D], F32R, tag="at")
            nc.vector.tensor_copy(at[:], atp[:])
            C = []
            for m in range(2):
                bp = psum_small.tile([128, D], F32, tag="ps_small")
                nc.tensor.matmul(
                    bp[:],
                    at[:, m * 128 : (m + 1) * 128],
                    wp_sb[:, l, :],
                    start=True,
                    stop=True,
                )
                cm = small_pool.tile([128, D], F32R, tag=f"c{m}")
                nc.vector.tensor_add(
                    cm[:], bp[:], idz[:, 128 - m * 128 : 384 - m * 128]
                )
                C.append(cm)

            # ---- update: x_new^T[d2,s] = C^T x^T  (fp32r, N=512) ----
            nsc = NSC // 2 if last else NSC
            if last:
                new_f = [
                    out_pool.tile([128, H], F32, tag=f"os{m}", name=f"os{m}")
                    for m in range(2)
                ]
                new_b = None
            else:
                new_f = [
                    xf_pool.tile([128, S], F32R, tag=f"xf{t}", name=f"xf_l{l}_{t}")
                    for t in range(2)
                ]
                new_b = [
                    xb_pool.tile([128, S], BF16, tag=f"xb{t}", name=f"xb_l{l}_{t}")
                    for t in range(2)
                ]
            for m in range(2):
                for g in range(nsc // 2):  # [128,1024] psum = 2 banks, 2 matmul groups
                    pu = psum_upd.tile([128, 1024], F32)
                    for hlf in range(2):
                        scol = (g * 2 + hlf) * 512
                        for t in range(2):
                            nc.tensor.matmul(
                                pu[:, hlf * 512 : (hlf + 1) * 512],
                                C[t][:, m * 128 : (m + 1) * 128],
                                cur_f[t][:, scol : scol + 512],
                                start=(t == 0),
                                stop=(t == 1),
                            )
                    col = g * 1024
                    nc.vector.tensor_copy(new_f[m][:, col : col + 1024], pu[:])
                    if not last:
                        nc.scalar.copy(new_b[m][:, col : col + 1024], pu[:])
            cur_f, cur_b = new_f, new_b

        # ---- output: first H columns of final x^T ----
        for m in range(2):
            for g in range(2):
                nc.sync.dma_start(
                    out[m, :, g * 1024 : (g + 1) * 1024],
                    cur_f[m][:, g * 1024 : (g + 1) * 1024],
                )


def _build():
    nc = bacc.Bacc(
        "TRN2", target_bir_lowering=False, debug=False, num_devices=8
    )
    xT = nc.dram_tensor("xT", [2, 128, S], F32, kind="ExternalInput")
    xTb = nc.dram_tensor("xTb", [2, 128, S], BF16, kind="ExternalInput")
    wkv = nc.dram_tensor("wkv", [L, 2, 128, 2 * KD], BF16, kind="ExternalInput")
    wqT = nc.dram_tensor("wqT", [L, KD, D], F32, kind="ExternalInput")
    wp = nc.dram_tensor("wp", [L, VD, D], F32, kind="ExternalInput")
    out = nc.dram_tensor("out", [2, 128, H], F32, kind="ExternalOutput")
    with tile.TileContext(nc) as tc:
        _emit(tc, nc, xT[:], xTb[:], wkv[:], wqT[:], wp[:], out[:])
    nc.compile()
    return nc


def get_nc():
    if "nc" not in _CACHE:
        _CACHE["nc"] = _build()
    return _CACHE["nc"]


def make_in_maps(x, Wq, Wk, Wv, Wp):
    bf16 = ml_dtypes.bfloat16
    wkv_np = np.empty((L, 2, 128, 2 * KD), dtype=bf16)
    for l in range(L):
        for t in range(2):
            wkv_np[l, t, :, :KD] = Wk[l][t * 128 : (t + 1) * 128, :].astype(bf16)
            wkv_np[l, t, :, KD:] = Wv[l][t * 128 : (t + 1) * 128, :].astype(bf16)
    wqT_np = np.ascontiguousarray(Wq.transpose(0, 2, 1)).astype(np.float32)
    wp_np = np.ascontiguousarray(Wp).astype(np.float32)

    in_maps = []
    for core in range(8):
        b, h = core // 2, core % 2
        xb = x[b] if h == 0 else np.roll(x[b], -H, axis=0)
        xTc = np.ascontiguousarray(xb.T).reshape(2, 128, S).astype(np.float32)
        in_maps.append(
            {
                "xT": xTc,
                "xTb": xTc.astype(bf16),
                "wkv": wkv_np,
                "wqT": wqT_np,
                "wp": wp_np,
            }
        )
    return in_maps


def assemble(results):
    out = np.empty((B, S, D), dtype=np.float32)
    for b in range(B):
        halves = []
        for h in range(2):
            o = results[2 * b + h]["out"]  # [2, 128, H]
            halves.append(o.reshape(D, H))
        out[b] = np.concatenate(halves, axis=1).T
    return out


def run(x, Wq, Wk, Wv, Wp, **spmd_kwargs):
    nc = get_nc()
    in_maps = make_in_maps(x, Wq, Wk, Wv, Wp)
    res = run_bass_kernel_spmd(nc, in_maps, core_ids=list(range(8)), **spmd_kwargs)
    return assemble(res.results), res


def kernel(x, Wq, Wk, Wv, Wp):
    out, _ = run(x, Wq, Wk, Wv, Wp)
    return out


# revision 15
# speedup vs baseline: 1.3336x; 1.3336x over previous
"""Trainium2 Bass kernel for nn_LSA_Model (4-layer linear self-attention).

Math: each layer computes  x <- x + ((x Wq)(K^T V)) Wp  with K = x Wk, V = x Wv.
Since there is no softmax, associativity lets us avoid the [S,S] score matrix:
    KV = K^T V                  [KD, VD]   (contraction over full sequence)
    A^T = KV^T Wq^T             [VD, D]
    B  = A Wp,  C = I + B       [D, D]
    x_new = x C   (residual folded into C via the identity)

Sharding: each batch element's forward is replicated on a pair of cores
(B=4 -> 8 cores).  Inter-core collectives have a ~10us floor per call which
would dominate, so the pair does redundant compute; each core of a pair
writes a different half of the output sequence.  The second core of a pair
receives the sequence ROLLED by S/2 (the model is permutation-equivariant
over the sequence axis), so a single static NEFF can always emit its first
S/2 columns.

Layouts: on-chip state is x^T [D, S] (d-major, two 128-partition tiles) in
bf16; matmuls accumulate in fp32 PSUM.  The tiny A/B chain runs in fp32r.
"""

import numpy as np
import ml_dtypes

import concourse.bacc as bacc
import concourse.mybir as mybir
import concourse.tile as tile
from concourse import masks
from concourse.bass_utils import run_bass_kernel_spmd

L, B, S, D, KD, VD = 4, 4, 4096, 256, 64, 64
H = S // 2          # output half per core
NCH = S // 128      # 32 projection chunks
NSC = S // 512      # 8 update chunks of 512
F32 = mybir.dt.float32
F32R = mybir.dt.float32r
BF16 = mybir.dt.bfloat16

_CACHE = {}


def _emit(tc, nc, xTb, wkv, wqT, wp, out):
    with (
        tc.tile_pool(name="singles", bufs=1) as singles,
        tc.tile_pool(name="xb", bufs=2) as xb_pool,
        tc.tile_pool(name="kvs", bufs=4) as kv_pool,
        tc.tile_pool(name="small", bufs=2) as small_pool,
        tc.tile_pool(name="pp", bufs=3, space="PSUM") as psum_proj,
        tc.tile_pool(name="pk", bufs=1, space="PSUM") as psum_kv,
        tc.tile_pool(name="ps", bufs=1, space="PSUM") as psum_small,
        tc.tile_pool(name="pu", bufs=3, space="PSUM") as psum_upd,
    ):
        # ---- PE warmup: release the HAM clock gate while DMAs stream ----
        warm_w = singles.tile([128, KD], BF16)
        warm_r = singles.tile([128, 512], BF16)
        nc.gpsimd.memset(warm_w[:], 0.0)
        nc.gpsimd.memset(warm_r[:], 0.0)
        warm_ps = psum_kv.tile([KD, 512], F32, tag="pk", name="warm_ps")
        for w in range(8):
            nc.tensor.matmul(warm_ps[:], warm_w[:], warm_r[:], start=True, stop=True)

        # ---- weights to SBUF (host-packed layouts -> single DMAs) ----
        wkv_sb = singles.tile([128, L, 2, 2 * KD], BF16)
        nc.sync.dma_start(wkv_sb[:], wkv[:])
        # fp32r matmul operands must be produced (rounded) by a compute engine,
        # so fp32 weights bounce through a staging tile into f32r tiles.
        wq_stage = singles.tile([KD, L, D], F32)
        nc.gpsimd.dma_start(wq_stage[:], wqT[:])
        wqT_sb = singles.tile([KD, L, D], F32R)
        nc.vector.tensor_copy(wqT_sb[:], wq_stage[:])
        wp_stage = singles.tile([VD, L, D], F32)
        nc.gpsimd.dma_start(wp_stage[:], wp[:])
        wp_sb = singles.tile([VD, L, D], F32R)
        nc.vector.tensor_copy(wp_sb[:], wp_stage[:])

        # identity staging: idz = [0(128) | I(128) | 0(128)]
        # C_m = B_m + idz[:, 128-m*128 : 384-m*128]
        idz = singles.tile([128, 384], F32)
        nc.gpsimd.memset(idz[:], 0.0)
        masks.make_identity(nc, idz[:, 128:256], nomemset=True)

        # ---- x state (bf16) to SBUF, t-interleaved so chunk 0 lands first ----
        cur_b = [
            xb_pool.tile([128, S], BF16, tag=f"xb{t}", name=f"xb_in{t}")
            for t in range(2)
        ]
        for g in range(4):
            for t in range(2):
                nc.sync.dma_start(
                    cur_b[t][:, g * 1024 : (g + 1) * 1024],
                    xTb[t, :, g * 1024 : (g + 1) * 1024],
                )

        def _copy(k, out_ap, in_ap):
            # alternate PSUM->SBUF copies between DVE and ACT
            if k % 2 == 0:
                nc.vector.tensor_copy(out_ap, in_ap)
            else:
                nc.scalar.copy(out_ap, in_ap)

        ce = 0

        for l in range(L):
            last = l == L - 1

            # ---- K|V projection (bf16) + KV = K^T V accumulation ----
            kvacc = psum_kv.tile([KD, VD], F32, tag="pk", name=f"kvacc{l}")
            for q in range(NCH // 4):  # 8 psum tiles, 4 chunks each
                pp = psum_proj.tile([128, 512], F32)
                for i in range(4):
                    c = q * 4 + i
                    for t in range(2):
                        nc.tensor.matmul(
                            pp[:, i * 128 : (i + 1) * 128],
                            cur_b[t][:, c * 128 : (c + 1) * 128],
                            wkv_sb[:, l, t, :],
                            start=(t == 0),
                            stop=(t == 1),
                        )
                kvt = kv_pool.tile([128, 512], BF16)
                _copy(ce, kvt[:], pp[:])
                ce += 1
                for i in range(4):
                    c = q * 4 + i
                    nc.tensor.matmul(
                        kvacc[:],
                        kvt[:, i * 128 : i * 128 + KD],
                        kvt[:, i * 128 + KD : (i + 1) * 128],
                        start=(c == 0),
                        stop=(c == NCH - 1),
                    )
            kvb = small_pool.tile([KD, VD], F32R, tag="kvb")
            nc.scalar.copy(kvb[:], kvacc[:])

            # ---- A^T = KV^T Wq^T ;  B = A Wp ;  C = I + B (bf16) ----
            atp = psum_small.tile([VD, D], F32, tag="ps_small")
            nc.tensor.matmul(atp[:], kvb[:], wqT_sb[:, l, :], start=True, stop=True)
            at = small_pool.tile([VD, D], F32R, tag="at")
            nc.scalar.copy(at[:], atp[:])
            C = []
            for m in range(2):
                bp = psum_small.tile([128, D], F32, tag="ps_small")
                nc.tensor.matmul(
                    bp[:],
                    at[:, m * 128 : (m + 1) * 128],
                    wp_sb[:, l, :],
                    start=True,
                    stop=True,
                )
                cm = small_pool.tile([128, D], BF16, tag=f"c{m}")
                nc.vector.tensor_add(
                    cm[:], bp[:], idz[:, 128 - m * 128 : 384 - m * 128]
                )
                C.append(cm)

            # ---- update: x_new^T[d2,s] = C^T x^T  (bf16, N=512) ----
            nsc = NSC // 2 if last else NSC
            if last:
                ost = [
                    small_pool.tile([128, H], F32, tag=f"os{m}", name=f"os{m}")
                    for m in range(2)
                ]
            else:
                new_b = [
                    xb_pool.tile([128, S], BF16, tag=f"xb{t}", name=f"xb_l{l}_{t}")
                    for t in range(2)
                ]
            for m in range(2):
                for sc in range(nsc):
                    pu = psum_upd.tile([128, 512], F32)
                    scol = sc * 512
                    for t in range(2):
                        nc.tensor.matmul(
                            pu[:],
                            C[t][:, m * 128 : (m + 1) * 128],
                            cur_b[t][:, scol : scol + 512],
                            start=(t == 0),
                            stop=(t == 1),
                        )
                    if last:
                        _copy(ce, ost[m][:, scol : scol + 512], pu[:])
                        ce += 1
                        nc.sync.dma_start(
                            out[m, :, scol : scol + 512],
                            ost[m][:, scol : scol + 512],
                        )
                    else:
                        _copy(ce, new_b[m][:, scol : scol + 512], pu[:])
                        ce += 1
            if not last:
                cur_b = new_b


def _build():
    nc = bacc.Bacc("TRN2", target_bir_lowering=False, debug=False, num_devices=8)
    xTb = nc.dram_tensor("xTb", [2, 128, S], BF16, kind="ExternalInput")
    wkv = nc.dram_tensor("wkv", [128, L, 2, 2 * KD], BF16, kind="ExternalInput")
    wqT = nc.dram_tensor("wqT", [KD, L, D], F32, kind="ExternalInput")
    wp = nc.dram_tensor("wp", [VD, L, D], F32, kind="ExternalInput")
    out = nc.dram_tensor("out", [2, 128, H], F32, kind="ExternalOutput")
    with tile.TileContext(nc) as tc:
        _emit(tc, nc, xTb[:], wkv[:], wqT[:], wp[:], out[:])
    nc.compile()
    return nc


def get_nc():
    if "nc" not in _CACHE:
        _CACHE["nc"] = _build()
    return _CACHE["nc"]


def make_in_maps(x, Wq, Wk, Wv, Wp):
    bf16 = ml_dtypes.bfloat16
    wkv_np = np.empty((128, L, 2, 2 * KD), dtype=bf16)
    for l in range(L):
        for t in range(2):
            wkv_np[:, l, t, :KD] = Wk[l][t * 128 : (t + 1) * 128, :].astype(bf16)
            wkv_np[:, l, t, KD:] = Wv[l][t * 128 : (t + 1) * 128, :].astype(bf16)
    # wqT dram layout [KD, L, D] = Wq^T per layer; wp [VD, L, D]
    wqT_np = np.ascontiguousarray(Wq.transpose(2, 0, 1)).astype(np.float32)
    wp_np = np.ascontiguousarray(Wp.transpose(1, 0, 2)).astype(np.float32)

    in_maps = []
    for core in range(8):
        b, h = core // 2, core % 2
        xb = x[b] if h == 0 else np.roll(x[b], -H, axis=0)
        xTc = np.ascontiguousarray(xb.T).reshape(2, 128, S).astype(bf16)
        in_maps.append(
            {"xTb": xTc, "wkv": wkv_np, "wqT": wqT_np, "wp": wp_np}
        )
    return in_maps


def assemble(results):
    out = np.empty((B, S, D), dtype=np.float32)
    for b in range(B):
        halves = []
        for h in range(2):
            o = results[2 * b + h]["out"]  # [2, 128, H]
            halves.append(o.reshape(D, H))
        out[b] = np.concatenate(halves, axis=1).T
    return out


def run(x, Wq, Wk, Wv, Wp, **spmd_kwargs):
    nc = get_nc()
    in_maps = make_in_maps(x, Wq, Wk, Wv, Wp)
    res = run_bass_kernel_spmd(nc, in_maps, core_ids=list(range(8)), **spmd_kwargs)
    return assemble(res.results), res


def kernel(x, Wq, Wk, Wv, Wp):
    out, _ = run(x, Wq, Wk, Wv, Wp)
    return out
